# revision 47
# baseline (speedup 1.0000x reference)
"""Bass/Trainium2 kernel for batched cross-attention (nn_Attention).

Reference math (per batch element, B=8 sharded one-per-core):
    tmp1   = h @ W_b                  [S, D]
    scores = tmp1 @ b^T               [S, S]
    attn   = softmax(scores, -1)
    cxt    = attn @ b                 [S, D]

v12 — host-side layout prep + fp16 QK path, ACT-paced steady loop:
  The host passes b^T and h^T (fp16) and [b | ones] (bf16, device tile
  layout) directly, so the device never runs PE transposes or their
  PSUM->SBUF copies; the scalar engine runs nothing but the 128 exp
  instructions (the pacer: ~1000ns each measured, (1024+352)/1.2GHz
  model).  Per step g (phase k = g//32, s-tile si = g%32):
    - QK: scoresT[si, t-block k] = bT-tile^T @ t1T   (fp16, 2x512)
    - exp: one ACT instruction [128, 1024] PSUM->SBUF bf16, bias=-SHIFT
      (softmax is shift-invariant; score max ~91 would overflow fp32 exp)
    - cxt for step g-L: 8 accumulating matmuls consuming the attn tile
      exp'd L steps ago (bf16 stationary = fast LDWEIGHTS).
    - injected: 2 t1mm chunks per phase for the NEXT phase's tmp1^T
      (fp16 W @ h^T chunk -> PSUM -> DVE cast to fp16 SBUF).
  Denominators ride along as a ones-column in the rhs [b_bf16 | 1];
  accumulators are packed 3-per-PSUM-bank ([128,129] each; start=True
  only on the first write into each bank - start marks the whole 2KB
  bank pending-zero). Block epilogue for phases 0-2: DVE copies the acc
  banks to SBUF first (the next phase's start=True matmul waits on the
  banks), then reciprocal+mul at leisure; the last phase normalizes
  straight out of PSUM with recips first and the muls split ACT/DVE.
  Outputs leave in device [partition, tile, d] layout (host unshuffles).
"""

import sys

if "/opt/trn_rl_repo" not in sys.path:
    sys.path.insert(0, "/opt/trn_rl_repo")

import numpy as np

B = 8
S = 4096
D = 128
P = 128
NT = S // P          # 32 seq tiles
TB = 1024            # t-block width
NB = S // TB         # 4 t-blocks
TT = TB // P         # 8 t-tiles per block
QCHUNK = 512         # psum-bank-sized matmul output max (f32)
SHIFT = 48.0         # exp(s - SHIFT): keeps exp finite (score max ~91)
ACC_PACK = 3         # [128,129] accumulators packed per PSUM bank
LAG = 2              # steps between exp(g) and its cxt consumption

_GRAPH = None


def _build_graph():
    import concourse.mybir as mybir
    import concourse.tile as tile
    from concourse import bacc
    from concourse.masks import make_identity

    f32 = mybir.dt.float32
    f16 = mybir.dt.float16
    bf16 = mybir.dt.bfloat16
    Exp = mybir.ActivationFunctionType.Exp

    nc = bacc.Bacc()
    # b1/out use host-shuffled [partition, tile, d] layouts so every DMA is
    # contiguous per partition (fewest descriptors, fastest dispatch)
    bT_ext = nc.declare_dram_parameter("bT", [D, S], f16, isOutput=False)
    hT_ext = nc.declare_dram_parameter("hT", [D, S], f16, isOutput=False)
    w_ext = nc.declare_dram_parameter("W_b", [D, D], f16, isOutput=False)
    b1_ext = nc.declare_dram_parameter("b1", [P, NT * (D + 1)], bf16, isOutput=False)
    out_ext = nc.declare_dram_parameter("out", [P, NT * D], f32, isOutput=True)

    b1_pnd = b1_ext.rearrange("p (n d) -> p n d", d=D + 1)   # [128, 32, 129]
    out_pnd = out_ext.rearrange("p (n d) -> p n d", d=D)

    n_acc_tiles = (TT + ACC_PACK - 1) // ACC_PACK        # 3

    with tile.TileContext(nc) as tc:
        with (
            tc.tile_pool(name="const", bufs=1) as const_pool,
            tc.tile_pool(name="big", bufs=1) as big,
            tc.tile_pool(name="attn_pool", bufs=6) as attn_pool,
            tc.tile_pool(name="outp", bufs=2) as outp,
            tc.tile_pool(name="small", bufs=4) as small,
            tc.tile_pool(name="ps_sc", bufs=2, space="PSUM") as ps_sc,
            tc.tile_pool(name="ps_acc", bufs=1, space="PSUM") as ps_acc,
        ):
            W_sb = const_pool.tile([D, D], f16)
            nc.sync.dma_start(out=W_sb, in_=w_ext[:, :])
            shift_ap = const_pool.tile([P, 1], f32)
            nc.vector.memset(shift_ap, -SHIFT)
            ident = const_pool.tile([P, P], f32)
            make_identity(nc, ident)

            bT = big.tile([P, S], f16)
            hT = big.tile([P, S], f16)
            t1T = big.tile([P, S], f16)
            b1 = big.tile([P, NT, D + 1], bf16)

            # input DMAs, chunked and dispatched in consumption order; the
            # early window is HBM-delivery-bound, so order = deadlines.
            # Critical path is {W, hT[0:1024]} -> t1mm(0,1) -> QK(0) -> exp(0);
            # ACT (idle until the first exp) dispatches hT/b1c0 in parallel
            # with Sync, each as ONE DMA (per-DMA fixed latency ~1us), and
            # GPSIMD's software DGE takes the late-deadline hT chunks.
            nc.scalar.dma_start(out=hT[:, 0:1024], in_=hT_ext[:, 0:1024])
            NCH = 4
            CW = S // NCH                                 # 1024 cols / chunk
            CT = NT // NCH                                # 8 s-tiles / chunk
            nc.sync.dma_start(out=bT[:, 0:1024], in_=bT_ext[:, 0:1024])
            nc.scalar.dma_start(out=b1[:, 0:CT, :], in_=b1_pnd[:, 0:CT, :])
            for c in range(1, NCH):
                cw = slice(c * CW, (c + 1) * CW)
                ct = slice(c * CT, (c + 1) * CT)
                nc.sync.dma_start(out=bT[:, cw], in_=bT_ext[:, cw])
                nc.sync.dma_start(out=b1[:, ct, :], in_=b1_pnd[:, ct, :])

            def t1mm(c, gate=None, tag="tr", cast_on_act=False):
                t = ps_acc.tile([P, QCHUNK], f32, tag=tag, name=f"t1_{c}")
                if gate is not None:
                    # WAW-gate: a 1-col dummy write keyed on a mid-phase attn
                    # tile pins this t1mm at its intended schedule position.
                    # Otherwise the scheduler hoists all t1mm matmuls to the
                    # front of the in-order PE stream, where they both stall
                    # early QKs and drag the whole hT load into the already
                    # delivery-bound early DMA window.
                    nc.vector.tensor_copy(t[:, 0:1], gate[:, 0:1])
                nc.tensor.matmul(
                    t,
                    lhsT=W_sb,
                    rhs=hT[:, c * QCHUNK : (c + 1) * QCHUNK],
                    start=True,
                    stop=True,
                )
                if cast_on_act:
                    nc.scalar.copy(t1T[:, c * QCHUNK : (c + 1) * QCHUNK], t)
                else:
                    nc.vector.tensor_copy(t1T[:, c * QCHUNK : (c + 1) * QCHUNK], t)

            # --- prologue: warm ACT table + PE clock, then phase 0 inputs ---
            warm = small.tile([P, 1], f32, tag="warm")
            nc.scalar.activation(out=warm, in_=shift_ap, func=Exp)

            def warmup(i):
                # dummy transposes keep the PE busy while the first DMAs land
                # so the HAM clock gate / p-state ramp is released early
                wt = ps_acc.tile([P, P], f32, tag=f"acc{i % 3}", name="wrm")
                nc.tensor.transpose(wt, ident, ident)

            for i in range(4):
                warmup(i)
            t1mm(0)
            for i in range(3):
                warmup(i)
            # prologue-only: stage in a (still-free) acc bank and cast on the
            # (still-idle) ACT so t1mm(1) fully overlaps t1mm(0)'s cast
            t1mm(1, tag="acc1", cast_on_act=True)
            # the late hT chunks ride GPSIMD's software DGE, dispatched after
            # make_identity's gpsimd work so they don't delay the warmups
            for c in range(1, NCH):
                cw = slice(c * CW, (c + 1) * CW)
                nc.gpsimd.dma_start(out=hT[:, cw], in_=hT_ext[:, cw])
            for i in range(3):
                warmup(i)

            # --- steady loop ---
            ats_all = []
            cur = {"accs": None}

            def make_accs():
                return [
                    ps_acc.tile(
                        [P, ACC_PACK * (D + 1)], f32, tag=f"acc{a}", name=f"acc_{a}"
                    )
                    for a in range(n_acc_tiles)
                ]

            def block_epilogue(accs, tb):
                Copy = mybir.ActivationFunctionType.Copy
                o_big = outp.tile([P, TT, D], f32, tag="ot", name=f"o_big_{tb}")
                last = tb == NB - 1
                if not last:
                    # free the acc PSUM banks ASAP (the next phase's first
                    # start=True cxt matmul waits on them): one DVE copy per
                    # bank, then normalize from the SBUF staging copy
                    sb = outp.tile(
                        [P, n_acc_tiles * ACC_PACK * (D + 1)],
                        f32,
                        tag="accsb",
                        name=f"accsb_{tb}",
                    )
                    aw = ACC_PACK * (D + 1)
                    for a in range(n_acc_tiles):
                        nc.vector.tensor_copy(sb[:, a * aw : (a + 1) * aw], accs[a])
                    src = lambda tt: sb[
                        :,
                        (tt // ACC_PACK) * aw
                        + (tt % ACC_PACK) * (D + 1) : (tt // ACC_PACK) * aw
                        + (tt % ACC_PACK) * (D + 1)
                        + D
                        + 1,
                    ]
                else:
                    src = lambda tt: accs[tt // ACC_PACK][
                        :,
                        (tt % ACC_PACK) * (D + 1) : (tt % ACC_PACK) * (D + 1) + D + 1,
                    ]
                if last:
                    # tail latency matters: all recips first (DVE), then the
                    # muls split across the now-idle ACT and DVE in parallel
                    recips = []
                    for tt in range(TT):
                        rc = small.tile([P, 1], f32, tag=f"rc{tt % 4}", name=f"r_{tt}")
                        nc.vector.reciprocal(rc, src(tt)[:, D : D + 1])
                        recips.append(rc)
                    for tt in range(TT):
                        if tt % 2 == 0:
                            nc.scalar.activation(
                                out=o_big[:, tt, :],
                                in_=src(tt)[:, 0:D],
                                func=Copy,
                                scale=recips[tt],
                            )
                        else:
                            nc.vector.tensor_scalar_mul(
                                o_big[:, tt, :], src(tt)[:, 0:D], recips[tt]
                            )
                        if tt == TT // 2 - 1:
                            nc.sync.dma_start(
                                out=out_pnd[:, tb * TT : tb * TT + TT // 2, :],
                                in_=o_big[:, 0 : TT // 2, :],
                            )
                else:
                    for tt in range(TT):
                        s = src(tt)
                        recip = small.tile(
                            [P, 1], f32, tag="recip", name=f"rc_{tb}_{tt}"
                        )
                        nc.vector.reciprocal(recip, s[:, D : D + 1])
                        nc.vector.tensor_scalar_mul(o_big[:, tt, :], s[:, 0:D], recip)
                        if tt == TT // 2 - 1:
                            nc.sync.dma_start(
                                out=out_pnd[:, tb * TT : tb * TT + TT // 2, :],
                                in_=o_big[:, 0 : TT // 2, :],
                            )
                nc.sync.dma_start(
                    out=out_pnd[:, tb * TT + TT // 2 : (tb + 1) * TT, :],
                    in_=o_big[:, TT // 2 :, :],
                )

            def lagged_cxt(gs):
                tgt = gs - LAG
                if tgt < 0:
                    return
                k2, s2 = divmod(tgt, NT)
                if s2 == 0:
                    cur["accs"] = make_accs()
                accs = cur["accs"]
                at = ats_all[tgt]
                for tt in range(TT):
                    acc = accs[tt // ACC_PACK]
                    off = (tt % ACC_PACK) * (D + 1)
                    # start=True marks the WHOLE 2KB psum bank pending-zero:
                    # issue it only on the first write into each bank.
                    nc.tensor.matmul(
                        acc[:, off : off + D + 1],
                        lhsT=at[:, tt * P : (tt + 1) * P],
                        rhs=b1[:, s2, :],
                        start=(s2 == 0 and tt % ACC_PACK == 0),
                        stop=(s2 == NT - 1),
                        skip_group_check=True,
                    )
                ats_all[tgt] = None  # release reference
                if s2 == NT - 1:
                    block_epilogue(accs, k2)

            gs = 0
            for k in range(NB):
                for si in range(NT):
                    # inject next phase's t1T chunks mid-phase, gated so the
                    # scheduler cannot hoist them into the prologue
                    if k < NB - 1:
                        if si == 17:
                            t1mm(2 * (k + 1), gate=ats_all[-1])
                        elif si == 19:
                            t1mm(2 * (k + 1) + 1, gate=ats_all[-1])

                    ps_s = ps_sc.tile([P, TB], f32, tag="sc")
                    for c in range(TB // QCHUNK):
                        nc.tensor.matmul(
                            ps_s[:, c * QCHUNK : (c + 1) * QCHUNK],
                            lhsT=bT[:, si * P : (si + 1) * P],
                            rhs=t1T[:, k * TB + c * QCHUNK : k * TB + (c + 1) * QCHUNK],
                            start=True,
                            stop=True,
                        )
                    at = attn_pool.tile([P, TB], bf16, tag="attn")
                    nc.scalar.activation(out=at, in_=ps_s, func=Exp, bias=shift_ap)
                    ats_all.append(at)
                    lagged_cxt(gs)
                    gs += 1
            for _ in range(LAG):
                lagged_cxt(gs)
                gs += 1

    return nc


def _get_graph():
    global _GRAPH
    if _GRAPH is None:
        _GRAPH = _build_graph()
        _GRAPH.finalize()
    return _GRAPH


def make_in_maps(b, h, W_b):
    import ml_dtypes

    b = np.asarray(b, dtype=np.float32)
    h = np.asarray(h, dtype=np.float32)
    W16 = np.ascontiguousarray(np.asarray(W_b, dtype=np.float32)).astype(np.float16)
    ones = np.ones((S, 1), dtype=np.float32)
    in_maps = []
    for i in range(B):
        b1 = np.concatenate([b[i], ones], axis=1).astype(ml_dtypes.bfloat16)
        # device-side layout: partition p holds s-positions p, p+128, ...
        b1 = np.ascontiguousarray(
            b1.reshape(NT, P, D + 1).transpose(1, 0, 2).reshape(P, NT * (D + 1))
        )
        in_maps.append(
            {
                "bT": np.ascontiguousarray(b[i].T).astype(np.float16),
                "hT": np.ascontiguousarray(h[i].T).astype(np.float16),
                "W_b": W16,
                "b1": b1,
            }
        )
    return in_maps


def unshuffle_out(o):
    # inverse of the device [p, n, d] output layout -> [S, D]
    return np.ascontiguousarray(
        np.asarray(o).reshape(P, NT, D).transpose(1, 0, 2).reshape(S, D)
    )


def kernel(b, h, W_b, **_ignored):
    nc = _get_graph()
    from concourse.bass_utils import run_bass_kernel_spmd

    in_maps = make_in_maps(b, h, W_b)
    res = run_bass_kernel_spmd(nc, in_maps, core_ids=list(range(B)))
    return np.stack([unshuffle_out(res.results[i]["out"]) for i in range(B)], axis=0)
